# revision 1
# baseline (speedup 1.0000x reference)
"""NGCF-style GNN forward on 8 Trainium2 NeuronCores.

Strategy: host precomputes dense [4096,4096] message matrices (edge
multiplicity folded in) sharded column-wise per core; device runs the
full layer stack with message-passing outputs AllGathered between
layers; the 128x41476 prediction layer is column-sharded (5185 classes
per core, padded to 41480).

All feature maps are kept in "T layout" [features on partitions, nodes
on free dim] except aggregation operands which live in normal layout
r-tiles. GCN biases are skipped (they cancel exactly inside BatchNorm).
pred_b is added on the host.
"""
import sys
sys.path.insert(0, '/opt/trn_rl_repo')
import numpy as np
from concourse import bass, tile, mybir
from concourse.bass_utils import run_bass_kernel_spmd
from concourse.vector_clock import ScopedClock
from concourse.tile_clock_wait import TileClockWait  # noqa: F401

AF = mybir.ActivationFunctionType
ALU = mybir.AluOpType
AX = mybir.AxisListType
FP32 = mybir.dt.float32

N = 4096
NCORES = 8
CH = 512            # nodes per core (message-pass column shard)
NT = N // 128       # 32 node r-tiles
NCLS = 41476
NPAD = 41480
CSL = NPAD // NCORES  # 5185 classes per core
BN_EPS = 1e-5
RG = [list(range(NCORES))]


# ---- workaround: this walrus build rejects instructions with >1 sync-wait;
# TileContext's final drain aggregates one wait per semaphore, so split them
# across single-wait SP nops.
def _patched_drain_and_barrier(self, tick_clock, wait_clock):
    nc = self.nc
    probe = nc.sync.nop(nofuse=True, hint="drain_wait_split").ins
    wait_clock.add_sem_waits(probe, ScopedClock({None: tick_clock.global_clock}))
    waits = list(probe.sync_info.on_wait) if probe.sync_info is not None else []
    if probe.sync_info is not None and len(waits) > 1:
        probe.sync_info = mybir.SyncInfo(on_wait=waits[:1], on_update=[])
        for w in waits[1:]:
            extra = nc.sync.nop(nofuse=True, hint="drain_wait_split").ins
            extra.sync_info = mybir.SyncInfo(on_wait=[w], on_update=[])
    nc.sync.drain()
    nc.all_engine_barrier()
    popped = nc._tile_sem_poison_stack.pop()
    assert popped is self._sem_poison
    nc.clear_and_free_semaphores(list(self.sems.allocated().values()))
    nc.all_engine_barrier()


tile.TileContext._drain_and_barrier = _patched_drain_and_barrier


# Same walrus limitation for mid-program instructions: during lowering,
# instructions are committed in final order, so extra waits can be peeled
# onto same-engine nops emitted just before the carrying instruction.
_orig_commit_and_lower = tile.TileContext._commit_and_lower


def _patched_commit_and_lower(self, inst, original_block, old_bb_map, bb_to_exit_bb):
    si = getattr(inst, "sync_info", None)
    eng_map = self.nc.engines
    if (si is not None and len(si.on_wait) > 1
            and type(inst).__module__.startswith("bass_rust")
            and inst.engine in eng_map):
        waits = list(si.on_wait)
        eng = eng_map[inst.engine]
        for w in waits[:-1]:
            nop_ins = eng.nop(nofuse=True, hint="wait_split").ins
            nop_ins.sync_info = mybir.SyncInfo(on_wait=[w], on_update=[])
        inst.sync_info = mybir.SyncInfo(on_wait=waits[-1:],
                                        on_update=list(si.on_update))
    return _orig_commit_and_lower(self, inst, original_block, old_bb_map,
                                  bb_to_exit_bb)


tile.TileContext._commit_and_lower = _patched_commit_and_lower


def _batch_norm(nc, bn_pool, mt, scratch, g_col, b_col, inv_n):
    """Per-partition BN stats over the free dim of mt [128, n].
    Returns (s, bp) [128,1] APs so caller applies relu(s*x + bp)."""
    mu_raw = bn_pool.tile([128, 1], FP32, name="mu_raw", bufs=2)
    nc.vector.reduce_sum(mu_raw[:], mt, axis=AX.X)
    sumsq = bn_pool.tile([128, 1], FP32, name="sumsq", bufs=2)
    nc.vector.scalar_tensor_tensor(scratch, mt, 1.0, mt, ALU.bypass, ALU.mult,
                                   accum_out=sumsq[:])
    mu = bn_pool.tile([128, 1], FP32, name="mu", bufs=2)
    nc.vector.tensor_scalar_mul(mu[:], mu_raw[:], inv_n)
    msq = bn_pool.tile([128, 1], FP32, name="msq", bufs=2)
    nc.vector.tensor_tensor(msq[:], mu[:], mu[:], ALU.mult)
    var = bn_pool.tile([128, 1], FP32, name="var", bufs=2)
    nc.vector.scalar_tensor_tensor(var[:], sumsq[:], inv_n, msq[:],
                                   ALU.mult, ALU.subtract)
    nc.vector.tensor_scalar_add(var[:], var[:], BN_EPS)
    std = bn_pool.tile([128, 1], FP32, name="std", bufs=2)
    nc.scalar.activation(std[:], var[:], AF.Sqrt)
    rinv = bn_pool.tile([128, 1], FP32, name="rinv", bufs=2)
    nc.vector.reciprocal(rinv[:], std[:])
    s = bn_pool.tile([128, 1], FP32, name="s", bufs=2)
    nc.vector.tensor_tensor(s[:], g_col, rinv[:], ALU.mult)
    sm = bn_pool.tile([128, 1], FP32, name="sm", bufs=2)
    nc.vector.tensor_tensor(sm[:], s[:], mu[:], ALU.mult)
    bp = bn_pool.tile([128, 1], FP32, name="bp", bufs=2)
    nc.vector.tensor_tensor(bp[:], b_col, sm[:], ALU.subtract)
    return s, bp


def build_program():
    nc = bass.Bass(num_devices=NCORES)

    def ein(name, shape):
        return nc.dram_tensor(name, shape, FP32, kind="ExternalInput")

    d_xin = ein("x_inT", [128, N])
    d_w1 = ein("w1", [128, 1024])
    d_b1 = ein("b1", [1024, 1])
    d_w2 = ein("w2", [1024, 512])
    d_b2 = ein("b2", [512, 1])
    d_gw1 = ein("gcn_w1", [512, 256])
    d_bn1g = ein("bn1_g", [256, 1])
    d_bn1b = ein("bn1_b", [256, 1])
    d_gw2 = ein("gcn_w2", [256, 128])
    d_bn2g = ein("bn2_g", [128, 1])
    d_bn2b = ein("bn2_b", [128, 1])
    d_swl = ein("sage_wl", [128, 128])
    d_sbl = ein("sage_bl", [128, 1])
    d_swr = ein("sage_wr", [128, 128])
    d_cw0 = ein("cheb_w0", [128, 128])
    d_cw1 = ein("cheb_w1", [128, 128])
    d_cb = ein("cheb_b", [128, 1])
    d_gwva1 = ein("gwva1", [128, 129])
    d_vd1 = ein("vd1", [128, 1])
    d_g1b = ein("g1b", [128, 1])
    d_gwva2 = ein("gwva2", [128, 129])
    d_vd2 = ein("vd2", [128, 1])
    d_g2b = ein("g2b", [128, 1])
    d_agcn = ein("a_gcn", [N, CH])
    d_asage = ein("a_sage", [N, CH])
    d_acheb = ein("a_cheb", [N, CH])
    d_mgat = ein("m_gat", [N, CH])
    d_pw = ein("pred_w", [128, CSL])
    d_scores = nc.dram_tensor("scores", [N, CSL], FP32, kind="ExternalOutput")

    # collective bounce buffers (internal DRAM; outputs in shared space)
    cc_in = {}
    cc_out = {}
    for tag, rows in [("gcn1", 256), ("gcn2", 128), ("sage", 128),
                      ("cheb", 128), ("gat1", 128), ("gat2", 128)]:
        cc_in[tag] = nc.dram_tensor(f"ccin_{tag}", [rows, CH], FP32)
        cc_out[tag] = nc.dram_tensor(f"ccout_{tag}", [NCORES * rows, CH], FP32,
                                     addr_space="Shared")

    with tile.TileContext(nc) as tc:
        with (
            tc.tile_pool(name="wts", bufs=1) as wp,
            tc.tile_pool(name="big", bufs=1) as bp_,
            tc.tile_pool(name="aux", bufs=1) as ax,
            tc.tile_pool(name="bn", bufs=1) as bnp,
            tc.tile_pool(name="astream", bufs=4) as asp,
        ):
            # ---- persistent SBUF arenas
            t_h2 = bp_.tile([128, 16384], FP32, name="t_h2")
            t_b2 = bp_.tile([128, 8192], FP32, name="t_b2")
            t_b3 = bp_.tile([128, 8192], FP32, name="t_b3")
            cc0 = ax.tile([128, CH], FP32, name="cc0")
            cc1 = ax.tile([128, CH], FP32, name="cc1")
            loc0 = ax.tile([128, CH], FP32, name="loc0")
            adb = ax.tile([128, CH], FP32, name="adb")
            a_s_sb = ax.tile([128, NT], FP32, name="a_s_sb")
            ad_row = ax.tile([1, CH], FP32, name="ad_row")
            rec_row = ax.tile([1, CH], FP32, name="rec_row")
            ones_row = ax.tile([1, 128], FP32, name="ones_row")
            ones_col = ax.tile([128, 1], FP32, name="ones_col")
            nc.vector.memset(ones_row[:], 1.0)
            nc.vector.memset(ones_col[:], 1.0)

            # ---- weight loads
            w1_sb = wp.tile([128, 1024], FP32, name="w1_sb")
            nc.sync.dma_start(w1_sb[:], d_w1[:])
            b1_sb = wp.tile([128, 8], FP32, name="b1_sb")
            for t in range(8):
                nc.sync.dma_start(b1_sb[:, t:t + 1], d_b1[128 * t:128 * (t + 1), :])
            w2_sb = t_b3[:, 4096:8192]
            for k in range(8):
                nc.sync.dma_start(w2_sb[:, 512 * k:512 * (k + 1)],
                                  d_w2[128 * k:128 * (k + 1), :])
            b2_sb = wp.tile([128, 4], FP32, name="b2_sb")
            for t in range(4):
                nc.sync.dma_start(b2_sb[:, t:t + 1], d_b2[128 * t:128 * (t + 1), :])
            gw1_sb = wp.tile([128, 1024], FP32, name="gw1_sb")
            for k in range(4):
                nc.sync.dma_start(gw1_sb[:, 256 * k:256 * (k + 1)],
                                  d_gw1[128 * k:128 * (k + 1), :])
            gw2_sb = wp.tile([128, 256], FP32, name="gw2_sb")
            for k in range(2):
                nc.sync.dma_start(gw2_sb[:, 128 * k:128 * (k + 1)],
                                  d_gw2[128 * k:128 * (k + 1), :])
            bn1g_sb = wp.tile([128, 2], FP32, name="bn1g_sb")
            bn1b_sb = wp.tile([128, 2], FP32, name="bn1b_sb")
            for t in range(2):
                nc.sync.dma_start(bn1g_sb[:, t:t + 1], d_bn1g[128 * t:128 * (t + 1), :])
                nc.sync.dma_start(bn1b_sb[:, t:t + 1], d_bn1b[128 * t:128 * (t + 1), :])
            bn2g_sb = wp.tile([128, 1], FP32, name="bn2g_sb")
            nc.sync.dma_start(bn2g_sb[:], d_bn2g[:])
            bn2b_sb = wp.tile([128, 1], FP32, name="bn2b_sb")
            nc.sync.dma_start(bn2b_sb[:], d_bn2b[:])
            swl_sb = wp.tile([128, 128], FP32, name="swl_sb")
            nc.sync.dma_start(swl_sb[:], d_swl[:])
            swr_sb = wp.tile([128, 128], FP32, name="swr_sb")
            nc.sync.dma_start(swr_sb[:], d_swr[:])
            sbl_sb = wp.tile([128, 1], FP32, name="sbl_sb")
            nc.sync.dma_start(sbl_sb[:], d_sbl[:])
            cw0_sb = wp.tile([128, 128], FP32, name="cw0_sb")
            nc.sync.dma_start(cw0_sb[:], d_cw0[:])
            cw1_sb = wp.tile([128, 128], FP32, name="cw1_sb")
            nc.sync.dma_start(cw1_sb[:], d_cw1[:])
            cb_sb = wp.tile([128, 1], FP32, name="cb_sb")
            nc.sync.dma_start(cb_sb[:], d_cb[:])
            gwva1_sb = wp.tile([128, 129], FP32, name="gwva1_sb")
            nc.sync.dma_start(gwva1_sb[:], d_gwva1[:])
            vd1_sb = wp.tile([128, 1], FP32, name="vd1_sb")
            nc.sync.dma_start(vd1_sb[:], d_vd1[:])
            g1b_sb = wp.tile([128, 1], FP32, name="g1b_sb")
            nc.sync.dma_start(g1b_sb[:], d_g1b[:])
            gwva2_sb = wp.tile([128, 129], FP32, name="gwva2_sb")
            nc.sync.dma_start(gwva2_sb[:], d_gwva2[:])
            vd2_sb = wp.tile([128, 1], FP32, name="vd2_sb")
            nc.sync.dma_start(vd2_sb[:], d_vd2[:])
            g2b_sb = wp.tile([128, 1], FP32, name="g2b_sb")
            nc.sync.dma_start(g2b_sb[:], d_g2b[:])

            x_inT = t_b3[:, 0:4096]
            nc.sync.dma_start(x_inT, d_xin[:])

            # ============ MLP: x_inT -> h2T (T layout, [512f, 4096n]) ========
            with tc.tile_pool(name="mlp_ps", bufs=2, space="PSUM") as mp:
                for j in range(8):
                    h1_base = 4096 * (j % 2)
                    for t in range(8):
                        ps1 = mp.tile([128, 512], FP32, name="ps1", bufs=2)
                        nc.tensor.matmul(ps1[:], w1_sb[:, 128 * t:128 * (t + 1)],
                                         x_inT[:, 512 * j:512 * (j + 1)],
                                         start=True, stop=True)
                        nc.scalar.activation(
                            t_b2[:, h1_base + 512 * t:h1_base + 512 * (t + 1)],
                            ps1[:], AF.Relu, bias=b1_sb[:, t:t + 1])
                    for f2 in range(4):
                        ps2 = mp.tile([128, 512], FP32, name="ps2", bufs=2)
                        for k in range(8):
                            nc.tensor.matmul(
                                ps2[:],
                                w2_sb[:, 512 * k + 128 * f2:512 * k + 128 * f2 + 128],
                                t_b2[:, h1_base + 512 * k:h1_base + 512 * (k + 1)],
                                start=(k == 0), stop=(k == 7))
                        nc.scalar.activation(
                            t_h2[:, 4096 * f2 + 512 * j:4096 * f2 + 512 * (j + 1)],
                            ps2[:], AF.Relu, bias=b2_sb[:, f2:f2 + 1])

            # ============ GCN1 feature: h_g1 [n,256] in t_b2 ================
            with tc.tile_pool(name="g1f_ps", bufs=2, space="PSUM") as gp:
                for rt in range(NT):
                    psg = gp.tile([128, 256], FP32, name="psg", bufs=2)
                    for k in range(4):
                        nc.tensor.matmul(
                            psg[:], t_h2[:, 4096 * k + 128 * rt:4096 * k + 128 * rt + 128],
                            gw1_sb[:, 256 * k:256 * (k + 1)],
                            start=(k == 0), stop=(k == 3))
                    nc.vector.tensor_copy(t_b2[:, 256 * rt:256 * (rt + 1)], psg[:])

            # ============ GCN1 message (local chunk) + AllGather ============
            with tc.tile_pool(name="g1m_ps", bufs=1, space="PSUM") as gp:
                acc0 = gp.tile([128, 512], FP32, name="acc0")
                acc1 = gp.tile([128, 512], FP32, name="acc1")
                for rt in range(NT):
                    a_t = asp.tile([128, 512], FP32, name="a_t", bufs=4)
                    nc.sync.dma_start(a_t[:], d_agcn[128 * rt:128 * (rt + 1), :])
                    nc.tensor.matmul(acc0[:], t_b2[:, 256 * rt:256 * rt + 128], a_t[:],
                                     start=(rt == 0), stop=(rt == NT - 1))
                    nc.tensor.matmul(acc1[:], t_b2[:, 256 * rt + 128:256 * rt + 256],
                                     a_t[:], start=(rt == 0), stop=(rt == NT - 1))
                nc.vector.tensor_copy(cc0[:], acc0[:])
                nc.vector.tensor_copy(cc1[:], acc1[:])
            nc.sync.dma_start(cc_in["gcn1"][0:128, :], cc0[:])
            nc.sync.dma_start(cc_in["gcn1"][128:256, :], cc1[:])
            nc.gpsimd.collective_compute(
                "AllGather", ALU.bypass, replica_groups=RG,
                ins=[cc_in["gcn1"][:].opt()], outs=[cc_out["gcn1"][:].opt()])
            for k in range(NCORES):
                nc.sync.dma_start(t_b3[:, 512 * k:512 * (k + 1)],
                                  cc_out["gcn1"][256 * k:256 * k + 128, :])
                nc.sync.dma_start(t_b3[:, 4096 + 512 * k:4096 + 512 * (k + 1)],
                                  cc_out["gcn1"][256 * k + 128:256 * (k + 1), :])

            # ============ BN1 + relu -> x3T (t_h2 blocks 1,2) ===============
            scratch = t_h2[:, 12288:16384]
            for t in range(2):
                mt = t_b3[:, 4096 * t:4096 * (t + 1)]
                s, bpc = _batch_norm(nc, bnp, mt, scratch,
                                     bn1g_sb[:, t:t + 1], bn1b_sb[:, t:t + 1],
                                     1.0 / N)
                nc.scalar.activation(t_h2[:, 4096 * (1 + t):4096 * (2 + t)], mt,
                                     AF.Relu, bias=bpc[:], scale=s[:])

            # ============ GCN2 feature: h_g2 [n,128] in t_b2 ================
            with tc.tile_pool(name="g2f_ps", bufs=2, space="PSUM") as gp:
                for rt in range(NT):
                    psg = gp.tile([128, 128], FP32, name="psg2", bufs=2)
                    for k in range(2):
                        nc.tensor.matmul(
                            psg[:],
                            t_h2[:, 4096 * (1 + k) + 128 * rt:4096 * (1 + k) + 128 * rt + 128],
                            gw2_sb[:, 128 * k:128 * (k + 1)],
                            start=(k == 0), stop=(k == 1))
                    nc.vector.tensor_copy(t_b2[:, 128 * rt:128 * (rt + 1)], psg[:])

            # ============ GCN2 message + AllGather ==========================
            with tc.tile_pool(name="g2m_ps", bufs=1, space="PSUM") as gp:
                accm = gp.tile([128, 512], FP32, name="accm")
                for rt in range(NT):
                    a_t = asp.tile([128, 512], FP32, name="a_t", bufs=4)
                    nc.sync.dma_start(a_t[:], d_agcn[128 * rt:128 * (rt + 1), :])
                    nc.tensor.matmul(accm[:], t_b2[:, 128 * rt:128 * (rt + 1)], a_t[:],
                                     start=(rt == 0), stop=(rt == NT - 1))
                nc.vector.tensor_copy(cc0[:], accm[:])
            nc.sync.dma_start(cc_in["gcn2"][:], cc0[:])
            nc.gpsimd.collective_compute(
                "AllGather", ALU.bypass, replica_groups=RG,
                ins=[cc_in["gcn2"][:].opt()], outs=[cc_out["gcn2"][:].opt()])
            for k in range(NCORES):
                nc.sync.dma_start(t_b3[:, 512 * k:512 * (k + 1)],
                                  cc_out["gcn2"][128 * k:128 * (k + 1), :])

            # ============ BN2 + relu -> x4T (t_b3 block 1) + local ==========
            mt_a = t_b3[:, 0:4096]
            s2, bp2 = _batch_norm(nc, bnp, mt_a, scratch,
                                  bn2g_sb[:, 0:1], bn2b_sb[:, 0:1], 1.0 / N)
            x4T = t_b3[:, 4096:8192]
            nc.scalar.activation(x4T, mt_a, AF.Relu, bias=bp2[:], scale=s2[:])
            nc.scalar.activation(loc0[:], cc0[:], AF.Relu, bias=bp2[:], scale=s2[:])

            # ============ SAGE ==============================================
            with tc.tile_pool(name="sage_ps", bufs=1, space="PSUM") as gp:
                for rt in range(NT):
                    psz = gp.tile([128, 128], FP32, name="psz", bufs=2)
                    nc.tensor.matmul(psz[:], x4T[:, 128 * rt:128 * (rt + 1)],
                                     swl_sb[:], start=True, stop=True)
                    nc.vector.tensor_copy(t_b2[:, 128 * rt:128 * (rt + 1)], psz[:])
                accs = gp.tile([128, 512], FP32, name="accs")
                for rt in range(NT):
                    a_t = asp.tile([128, 512], FP32, name="a_t", bufs=4)
                    nc.sync.dma_start(a_t[:], d_asage[128 * rt:128 * (rt + 1), :])
                    nc.tensor.matmul(accs[:], t_b2[:, 128 * rt:128 * (rt + 1)], a_t[:],
                                     start=(rt == 0), stop=False)
                nc.tensor.matmul(accs[:], swr_sb[:], loc0[:], start=False, stop=True)
                nc.scalar.activation(cc1[:], accs[:], AF.Relu, bias=sbl_sb[:])
            nc.sync.dma_start(cc_in["sage"][:], cc1[:])
            nc.gpsimd.collective_compute(
                "AllGather", ALU.bypass, replica_groups=RG,
                ins=[cc_in["sage"][:].opt()], outs=[cc_out["sage"][:].opt()])
            x5T = t_h2[:, 0:4096]
            for k in range(NCORES):
                nc.sync.dma_start(x5T[:, 512 * k:512 * (k + 1)],
                                  cc_out["sage"][128 * k:128 * (k + 1), :])

            # ============ Cheb ==============================================
            with tc.tile_pool(name="cheb_ps", bufs=1, space="PSUM") as gp:
                for rt in range(NT):
                    psz = gp.tile([128, 128], FP32, name="psz1", bufs=2)
                    nc.tensor.matmul(psz[:], x5T[:, 128 * rt:128 * (rt + 1)],
                                     cw1_sb[:], start=True, stop=True)
                    nc.vector.tensor_copy(t_b2[:, 4096 + 128 * rt:4096 + 128 * (rt + 1)],
                                          psz[:])
                accc = gp.tile([128, 512], FP32, name="accc")
                for rt in range(NT):
                    a_t = asp.tile([128, 512], FP32, name="a_t", bufs=4)
                    nc.sync.dma_start(a_t[:], d_acheb[128 * rt:128 * (rt + 1), :])
                    nc.tensor.matmul(accc[:], t_b2[:, 4096 + 128 * rt:4096 + 128 * (rt + 1)],
                                     a_t[:], start=(rt == 0), stop=False)
                nc.tensor.matmul(accc[:], cw0_sb[:], cc1[:], start=False, stop=True)
                nc.scalar.activation(cc0[:], accc[:], AF.Relu, bias=cb_sb[:])
            nc.sync.dma_start(cc_in["cheb"][:], cc0[:])
            nc.gpsimd.collective_compute(
                "AllGather", ALU.bypass, replica_groups=RG,
                ins=[cc_in["cheb"][:].opt()], outs=[cc_out["cheb"][:].opt()])
            x6T = t_b3[:, 0:4096]
            for k in range(NCORES):
                nc.sync.dma_start(x6T[:, 512 * k:512 * (k + 1)],
                                  cc_out["cheb"][128 * k:128 * (k + 1), :])

            # ============ GAT layers ========================================
            def gat_layer(xT, xloc, gwva_sb, vd_sb, gb_sb, h_base, out_loc, tag):
                with tc.tile_pool(name=f"{tag}_ps", bufs=1, space="PSUM") as gp:
                    for rt in range(NT):
                        psh = gp.tile([128, 129], FP32, name="psh", bufs=2)
                        nc.tensor.matmul(psh[:], xT[:, 128 * rt:128 * (rt + 1)],
                                         gwva_sb[:], start=True, stop=True)
                        nc.vector.tensor_copy(
                            t_b2[:, h_base + 128 * rt:h_base + 128 * (rt + 1)],
                            psh[:, 0:128])
                        nc.vector.tensor_copy(a_s_sb[:, rt:rt + 1], psh[:, 128:129])
                    psd = gp.tile([1, 512], FP32, name="psd")
                    nc.tensor.matmul(psd[:], vd_sb[:], xloc[:], start=True, stop=True)
                    nc.vector.tensor_copy(ad_row[:], psd[:])
                    psb = gp.tile([128, 512], FP32, name="psb")
                    nc.tensor.matmul(psb[:], ones_row[:], ad_row[:],
                                     start=True, stop=True)
                    nc.vector.tensor_copy(adb[:], psb[:])
                    accn = gp.tile([128, 512], FP32, name="accn")
                    accd = gp.tile([1, 512], FP32, name="accd")
                    for rt in range(NT):
                        e_t = ax.tile([128, 512], FP32, name="gat_et", bufs=2)
                        nc.scalar.activation(e_t[:], adb[:], AF.Lrelu,
                                             bias=a_s_sb[:, rt:rt + 1], alpha=0.2)
                        x_t = ax.tile([128, 512], FP32, name="gat_xt", bufs=2)
                        nc.scalar.activation(x_t[:], e_t[:], AF.Exp)
                        m_t = asp.tile([128, 512], FP32, name="a_t", bufs=4)
                        nc.sync.dma_start(m_t[:], d_mgat[128 * rt:128 * (rt + 1), :])
                        ab_t = ax.tile([128, 512], FP32, name="gat_ab", bufs=2)
                        nc.vector.tensor_tensor(ab_t[:], x_t[:], m_t[:], ALU.mult)
                        nc.tensor.matmul(accn[:],
                                         t_b2[:, h_base + 128 * rt:h_base + 128 * (rt + 1)],
                                         ab_t[:], start=(rt == 0), stop=(rt == NT - 1))
                        nc.tensor.matmul(accd[:], ones_col[:], ab_t[:],
                                         start=(rt == 0), stop=(rt == NT - 1))
                    nc.vector.reciprocal(rec_row[:], accd[:])
                    psr = gp.tile([128, 512], FP32, name="psr")
                    nc.tensor.matmul(psr[:], ones_row[:], rec_row[:],
                                     start=True, stop=True)
                    nc.vector.tensor_copy(adb[:], accn[:])
                    prod = ax.tile([128, 512], FP32, name="gat_ab", bufs=2)
                    nc.vector.tensor_tensor(prod[:], adb[:], psr[:], ALU.mult)
                    r_t = ax.tile([128, 512], FP32, name="gat_et", bufs=2)
                    nc.scalar.activation(r_t[:], prod[:], AF.Relu, bias=gb_sb[:])
                    m_n = ax.tile([128, 512], FP32, name="gat_xt", bufs=2)
                    nc.vector.tensor_scalar(m_n[:], prod[:], gb_sb[:], 0.0,
                                            ALU.add, ALU.min)
                    e2 = ax.tile([128, 512], FP32, name="gat_ab", bufs=2)
                    nc.scalar.activation(e2[:], m_n[:], AF.Exp)
                    nc.vector.scalar_tensor_tensor(out_loc[:], e2[:], -1.0, r_t[:],
                                                   ALU.add, ALU.add)

            gat_layer(x6T, cc0, gwva1_sb, vd1_sb, g1b_sb, 0, cc1, "gat1")
            nc.sync.dma_start(cc_in["gat1"][:], cc1[:])
            nc.gpsimd.collective_compute(
                "AllGather", ALU.bypass, replica_groups=RG,
                ins=[cc_in["gat1"][:].opt()], outs=[cc_out["gat1"][:].opt()])
            x7T = t_h2[:, 4096:8192]
            for k in range(NCORES):
                nc.sync.dma_start(x7T[:, 512 * k:512 * (k + 1)],
                                  cc_out["gat1"][128 * k:128 * (k + 1), :])

            gat_layer(x7T, cc1, gwva2_sb, vd2_sb, g2b_sb, 4096, cc0, "gat2")
            nc.sync.dma_start(cc_in["gat2"][:], cc0[:])
            nc.gpsimd.collective_compute(
                "AllGather", ALU.bypass, replica_groups=RG,
                ins=[cc_in["gat2"][:].opt()], outs=[cc_out["gat2"][:].opt()])
            x8T = t_b3[:, 4096:8192]
            for k in range(NCORES):
                nc.sync.dma_start(x8T[:, 512 * k:512 * (k + 1)],
                                  cc_out["gat2"][128 * k:128 * (k + 1), :])

            # ============ pred: scores[n, CSL] = x8 @ pred_w slice ==========
            pw_sb = t_h2[:, 0:CSL]
            for k in range(11):
                c0 = 512 * k
                cw = min(512, CSL - c0)
                nc.sync.dma_start(pw_sb[:, c0:c0 + cw], d_pw[:, c0:c0 + cw])
            chunks = [(512 * k, min(512, CSL - 512 * k)) for k in range(11)]
            cp_engines = [nc.vector, nc.scalar]
            with (
                tc.tile_pool(name="pred_ps", bufs=4, space="PSUM") as pp,
                tc.tile_pool(name="pred_out", bufs=4) as po,
            ):
                i = 0
                for nt in range(NT):
                    for (c0, cw) in chunks:
                        psp = pp.tile([128, 512], FP32, name="psp", bufs=4)
                        nc.tensor.matmul(psp[:, 0:cw], x8T[:, 128 * nt:128 * (nt + 1)],
                                         pw_sb[:, c0:c0 + cw], start=True, stop=True)
                        osb = po.tile([128, 512], FP32, name="osb", bufs=4)
                        eng = cp_engines[i % 2]
                        if eng is nc.scalar:
                            eng.copy(osb[:, 0:cw], psp[:, 0:cw])
                        else:
                            eng.tensor_copy(osb[:, 0:cw], psp[:, 0:cw])
                        nc.sync.dma_start(
                            d_scores[128 * nt:128 * (nt + 1), c0:c0 + cw],
                            osb[:, 0:cw])
                        i += 1
    return nc


_PROG = None


def _get_program():
    global _PROG
    if _PROG is None:
        _PROG = build_program()
    return _PROG


def host_prep(inputs):
    f32 = lambda a: np.ascontiguousarray(np.asarray(a), dtype=np.float32)
    ei = np.asarray(inputs["edge_index"])
    nx = np.asarray(inputs["node_x"])
    r = ei[0].astype(np.int64)
    c = ei[1].astype(np.int64)
    mult = np.bincount(r * N + c, minlength=N * N).reshape(N, N).astype(np.float32)

    deg = np.bincount(c, minlength=N).astype(np.float32) + 1.0
    dinv = deg ** -0.5
    a_gcn = mult * np.outer(dinv, dinv)
    idx = np.arange(N)
    a_gcn[idx, idx] += dinv * dinv

    cnt = np.bincount(c, minlength=N).astype(np.float32)
    a_sage = mult / np.maximum(cnt, 1.0)[None, :]

    deg0 = np.bincount(r, minlength=N).astype(np.float32)
    dinv0 = np.where(deg0 > 0, deg0 ** -0.5, 0.0).astype(np.float32)
    a_cheb = -(mult * np.outer(dinv0, dinv0))

    m_gat = mult
    m_gat[idx, idx] += 1.0

    ue = np.asarray(inputs["user_emb_w"])
    ie = np.asarray(inputs["item_emb_w"])
    x_in = np.concatenate([ue[nx[:, 0]], ie[nx[:, 1]]], axis=1)
    x_inT = f32(x_in.T)

    g1w = np.asarray(inputs["gat1_w"], dtype=np.float32)
    g2w = np.asarray(inputs["gat2_w"], dtype=np.float32)
    va1 = (g1w @ np.asarray(inputs["gat1_asrc"], dtype=np.float32)).reshape(128, 1)
    vd1 = (g1w @ np.asarray(inputs["gat1_adst"], dtype=np.float32)).reshape(128, 1)
    va2 = (g2w @ np.asarray(inputs["gat2_asrc"], dtype=np.float32)).reshape(128, 1)
    vd2 = (g2w @ np.asarray(inputs["gat2_adst"], dtype=np.float32)).reshape(128, 1)
    gwva1 = f32(np.concatenate([g1w, va1], axis=1))
    gwva2 = f32(np.concatenate([g2w, va2], axis=1))

    pw_pad = np.zeros((128, NPAD), dtype=np.float32)
    pw_pad[:, :NCLS] = np.asarray(inputs["pred_w"], dtype=np.float32)

    common = {
        "x_inT": x_inT,
        "w1": f32(inputs["mlp_w1"]),
        "b1": f32(np.asarray(inputs["mlp_b1"]).reshape(1024, 1)),
        "w2": f32(inputs["mlp_w2"]),
        "b2": f32(np.asarray(inputs["mlp_b2"]).reshape(512, 1)),
        "gcn_w1": f32(inputs["gcn_w1"]),
        "bn1_g": f32(np.asarray(inputs["bn1_g"]).reshape(256, 1)),
        "bn1_b": f32(np.asarray(inputs["bn1_b"]).reshape(256, 1)),
        "gcn_w2": f32(inputs["gcn_w2"]),
        "bn2_g": f32(np.asarray(inputs["bn2_g"]).reshape(128, 1)),
        "bn2_b": f32(np.asarray(inputs["bn2_b"]).reshape(128, 1)),
        "sage_wl": f32(inputs["sage_wl"]),
        "sage_bl": f32(np.asarray(inputs["sage_bl"]).reshape(128, 1)),
        "sage_wr": f32(inputs["sage_wr"]),
        "cheb_w0": f32(inputs["cheb_w0"]),
        "cheb_w1": f32(inputs["cheb_w1"]),
        "cheb_b": f32(np.asarray(inputs["cheb_b"]).reshape(128, 1)),
        "gwva1": gwva1, "vd1": f32(vd1),
        "g1b": f32(np.asarray(inputs["gat1_b"]).reshape(128, 1)),
        "gwva2": gwva2, "vd2": f32(vd2),
        "g2b": f32(np.asarray(inputs["gat2_b"]).reshape(128, 1)),
    }
    in_maps = []
    for k in range(NCORES):
        sl = slice(CH * k, CH * (k + 1))
        m = dict(common)
        m["a_gcn"] = np.ascontiguousarray(a_gcn[:, sl])
        m["a_sage"] = np.ascontiguousarray(a_sage[:, sl])
        m["a_cheb"] = np.ascontiguousarray(a_cheb[:, sl])
        m["m_gat"] = np.ascontiguousarray(m_gat[:, sl])
        m["pred_w"] = np.ascontiguousarray(pw_pad[:, CSL * k:CSL * (k + 1)])
        in_maps.append(m)
    return in_maps


def kernel(**inputs):
    in_maps = host_prep(inputs)
    nc = _get_program()
    res = run_bass_kernel_spmd(nc, in_maps, list(range(NCORES)))
    out = np.concatenate([res.results[k]["scores"] for k in range(NCORES)],
                         axis=1)[:, :NCLS]
    out = out + np.asarray(inputs["pred_b"], dtype=np.float32)[None, :]
    return np.ascontiguousarray(out, dtype=np.float32)



# revision 73
# speedup vs baseline: 1.9157x; 1.9157x over previous
"""NGCF-style GNN forward on 8 Trainium2 NeuronCores.

Strategy: host precomputes dense [4096,4096] message matrices (edge
multiplicity folded in) sharded column-wise per core; device runs the
full layer stack with message-passing outputs AllGathered between
layers; the 128x41476 prediction layer is column-sharded (5185 classes
per core, padded to 41480).

All matmuls run as float32r (full PE rate at free-dim >= 256, near-fp32
precision; fp32r needs even free dims, hence the gwva/vd/pred padding).
The MLP and GCN1 feature transform are node-sharded with an r-layout
AllGather of h_g1; the A-matrices stream through an 8-deep tile pool;
per-layer gather unloads and weight loads are single rearranged DMAs to
keep sequencer/HWDGE dispatch off the critical path. BN statistics run
split across DVE (sum) and ACT (sum-of-squares); GAT leaky-relu is
balanced across ACT and DVE; the prediction epilogue rotates PSUM->SBUF
casts across DVE/ACT into 2048-wide bf16 staging tiles flushed by wide
DMAs alternating the SP/ACT queues. Only the scores leave the chip as
bf16 (upcast + pred_b added on host). The four post-activation
gathers (SAGE, Cheb, GAT1, GAT2/x8) also travel as bf16 - quantized
once per layer boundary, cast back to fp32r on unload - halving their
collective payload; the two BatchNorm-feeding gathers stay fp32r since
BN amplifies pre-normalization quantization noise ~3x.

All feature maps are kept in "T layout" [features on partitions, nodes
on free dim] except aggregation operands which live in normal layout
r-tiles. GCN biases are skipped (they cancel exactly inside BatchNorm).
"""
import sys
sys.path.insert(0, '/opt/trn_rl_repo')
import numpy as np
import ml_dtypes
from concourse import bass, tile, mybir
from concourse.bass_utils import run_bass_kernel_spmd
from concourse.vector_clock import ScopedClock
from concourse.tile_clock_wait import TileClockWait  # noqa: F401

AF = mybir.ActivationFunctionType
ALU = mybir.AluOpType
AX = mybir.AxisListType
FP32 = mybir.dt.float32
FR = mybir.dt.float32r
BF16 = mybir.dt.bfloat16
NPBF = ml_dtypes.bfloat16

N = 4096
NCORES = 8
CH = 512            # nodes per core (message-pass column shard)
NT = N // 128       # 32 node r-tiles
NCLS = 41476
NPAD = 41480
CSL = NPAD // NCORES  # 5185 classes per core
BN_EPS = 1e-5
RG = [list(range(NCORES))]
DEBUG_TAPS = False


# ---- workaround: this walrus build rejects instructions with >1 sync-wait;
# TileContext's final drain aggregates one wait per semaphore, so split them
# across single-wait SP nops.
def _patched_drain_and_barrier(self, tick_clock, wait_clock):
    nc = self.nc
    probe = nc.sync.nop(nofuse=True, hint="drain_wait_split").ins
    wait_clock.add_sem_waits(probe, ScopedClock({None: tick_clock.global_clock}))
    waits = list(probe.sync_info.on_wait) if probe.sync_info is not None else []
    if probe.sync_info is not None and len(waits) > 1:
        probe.sync_info = mybir.SyncInfo(on_wait=waits[:1], on_update=[])
        for w in waits[1:]:
            extra = nc.sync.nop(nofuse=True, hint="drain_wait_split").ins
            extra.sync_info = mybir.SyncInfo(on_wait=[w], on_update=[])
    nc.sync.drain()
    nc.all_engine_barrier()
    popped = nc._tile_sem_poison_stack.pop()
    assert popped is self._sem_poison
    nc.clear_and_free_semaphores(list(self.sems.allocated().values()))
    nc.all_engine_barrier()


tile.TileContext._drain_and_barrier = _patched_drain_and_barrier


# Same walrus limitation for mid-program instructions: during lowering,
# instructions are committed in final order, so extra waits can be peeled
# onto same-engine nops emitted just before the carrying instruction.
_orig_commit_and_lower = tile.TileContext._commit_and_lower


def _patched_commit_and_lower(self, inst, original_block, old_bb_map, bb_to_exit_bb):
    si = getattr(inst, "sync_info", None)
    eng_map = self.nc.engines
    if (si is not None and len(si.on_wait) > 1
            and type(inst).__module__.startswith("bass_rust")
            and inst.engine in eng_map):
        waits = list(si.on_wait)
        eng = eng_map[inst.engine]
        for w in waits[:-1]:
            nop_ins = eng.nop(nofuse=True, hint="wait_split").ins
            nop_ins.sync_info = mybir.SyncInfo(on_wait=[w], on_update=[])
        inst.sync_info = mybir.SyncInfo(on_wait=waits[-1:],
                                        on_update=list(si.on_update))
    return _orig_commit_and_lower(self, inst, original_block, old_bb_map,
                                  bb_to_exit_bb)


tile.TileContext._commit_and_lower = _patched_commit_and_lower


def _batch_norm(nc, bn_pool, mt, scratch, g_col, b_col, inv_n):
    """Per-partition BN stats over the free dim of mt [128, n].
    Returns (s, bp) [128,1] fp32 APs so caller applies relu(s*x + bp).
    Sum runs on DVE while sum-of-squares runs on ACT so the two stat
    passes overlap."""
    mu_raw = bn_pool.tile([128, 1], FP32, name="mu_raw", bufs=2)
    nc.vector.reduce_sum(mu_raw[:], mt, axis=AX.X)
    sumsq = bn_pool.tile([128, 1], FP32, name="sumsq", bufs=2)
    nc.scalar.activation(scratch, mt, AF.Square, accum_out=sumsq[:])
    mu = bn_pool.tile([128, 1], FP32, name="mu", bufs=2)
    nc.vector.tensor_scalar_mul(mu[:], mu_raw[:], inv_n)
    msq = bn_pool.tile([128, 1], FP32, name="msq", bufs=2)
    nc.vector.tensor_tensor(msq[:], mu[:], mu[:], ALU.mult)
    var = bn_pool.tile([128, 1], FP32, name="var", bufs=2)
    nc.vector.scalar_tensor_tensor(var[:], sumsq[:], inv_n, msq[:],
                                   ALU.mult, ALU.subtract)
    nc.vector.tensor_scalar_add(var[:], var[:], BN_EPS)
    std = bn_pool.tile([128, 1], FP32, name="std", bufs=2)
    nc.scalar.activation(std[:], var[:], AF.Sqrt)
    rinv = bn_pool.tile([128, 1], FP32, name="rinv", bufs=2)
    nc.vector.reciprocal(rinv[:], std[:])
    s = bn_pool.tile([128, 1], FP32, name="s", bufs=2)
    nc.vector.tensor_tensor(s[:], g_col, rinv[:], ALU.mult)
    sm = bn_pool.tile([128, 1], FP32, name="sm", bufs=2)
    nc.vector.tensor_tensor(sm[:], s[:], mu[:], ALU.mult)
    bp = bn_pool.tile([128, 1], FP32, name="bp", bufs=2)
    nc.vector.tensor_tensor(bp[:], b_col, sm[:], ALU.subtract)
    return s, bp


def build_program():
    nc = bass.Bass(num_devices=NCORES)

    def ein(name, shape, dt=FR):
        return nc.dram_tensor(name, shape, dt, kind="ExternalInput")

    d_xin = ein("x_inT", [128, CH])
    d_w1 = ein("w1", [128, 1024])
    d_b1 = ein("b1", [1024, 1], FP32)
    d_w2 = ein("w2", [1024, 512])
    d_b2 = ein("b2", [512, 1], FP32)
    d_gw1 = ein("gcn_w1", [512, 256])
    d_bn1g = ein("bn1_g", [256, 1], FP32)
    d_bn1b = ein("bn1_b", [256, 1], FP32)
    d_gw2 = ein("gcn_w2", [256, 128])
    d_bn2g = ein("bn2_g", [128, 1], FP32)
    d_bn2b = ein("bn2_b", [128, 1], FP32)
    d_swl = ein("sage_wl", [128, 128])
    d_sbl = ein("sage_bl", [128, 1], FP32)
    d_swr = ein("sage_wr", [128, 128])
    d_cw0 = ein("cheb_w0", [128, 128])
    d_cw1 = ein("cheb_w1", [128, 128])
    d_cb = ein("cheb_b", [128, 1], FP32)
    d_gwva1 = ein("gwva1", [128, 130])
    d_vd1 = ein("vd1", [128, 2])
    d_g1b = ein("g1b", [128, 1], FP32)
    d_gwva2 = ein("gwva2", [128, 130])
    d_vd2 = ein("vd2", [128, 2])
    d_g2b = ein("g2b", [128, 1], FP32)
    d_agcn = ein("a_gcn", [N, CH])
    d_asage = ein("a_sage", [N, CH], BF16)
    d_acheb = ein("a_cheb", [N, CH], BF16)
    d_mgat = ein("m_gat", [N, CH], BF16)
    d_pw = ein("pred_w", [128, CSL + 1])
    d_scores = nc.dram_tensor("scores", [N, CSL], BF16, kind="ExternalOutput")
    d_dbg = {}
    if DEBUG_TAPS:
        for tag, w in [("h", 8192), ("x3", 8192), ("x4", 4096), ("x5", 4096),
                       ("x6", 4096), ("x7", 4096), ("x8", 4096)]:
            d_dbg[tag] = nc.dram_tensor(f"dbg_{tag}", [128, w], FR,
                                        kind="ExternalOutput")

    # collective bounce buffers (internal DRAM; outputs in shared space)
    cc_in = {}
    cc_out = {}
    BF_TAGS = ("sage", "cheb", "gat1", "gat2")
    for tag, rows in [("gcn1", 256), ("gcn2", 128), ("sage", 128),
                      ("cheb", 128), ("gat1", 128), ("gat2", 128)]:
        cdt = BF16 if tag in BF_TAGS else FR
        cc_in[tag] = nc.dram_tensor(f"ccin_{tag}", [rows, CH], cdt)
        cc_out[tag] = nc.dram_tensor(f"ccout_{tag}", [NCORES * rows, CH], cdt,
                                     addr_space="Shared")
    # r-layout h_g1 gather (sharded MLP): rows are global node ids
    cc_in_h = nc.dram_tensor("ccin_h", [CH, 256], FR)
    cc_out_h = nc.dram_tensor("ccout_h", [N, 256], FR, addr_space="Shared")

    mm = nc.tensor.matmul

    with tile.TileContext(nc) as tc:
        with (
            tc.tile_pool(name="wts", bufs=1) as wp,
            tc.tile_pool(name="big", bufs=1) as bp_,
            tc.tile_pool(name="aux", bufs=1) as ax,
            tc.tile_pool(name="bn", bufs=1) as bnp,
            tc.tile_pool(name="astream", bufs=4) as asp,
        ):
            # ---- persistent SBUF arenas (bf16)
            t_h2 = bp_.tile([128, 16384], FR, name="t_h2")
            t_b2 = bp_.tile([128, 8192], FR, name="t_b2")
            t_b3 = bp_.tile([128, 8192], FR, name="t_b3")
            cc0 = ax.tile([128, CH], FR, name="cc0")
            cc1 = ax.tile([128, CH], FR, name="cc1")
            loc0 = ax.tile([128, CH], FR, name="loc0")
            adb = ax.tile([128, CH], FP32, name="adb")
            a_s_sb = ax.tile([128, NT], FP32, name="a_s_sb")
            ad_row = ax.tile([1, CH], FP32, name="ad_row")
            rec_row = ax.tile([1, CH], FP32, name="rec_row")
            
            ones_row = ax.tile([1, 128], FP32, name="ones_row")
            ones_col2 = ax.tile([128, 2], FR, name="ones_col2")
            ones32c = ax.tile([128, 2], FP32, name="ones32c")
            nc.vector.memset(ones_row[:], 1.0)
            nc.vector.memset(ones32c[:], 1.0)
            nc.vector.tensor_copy(ones_col2[:], ones32c[:])
            ccb = ax.tile([128, CH], BF16, name="ccb")
            xb16 = ax.tile([128, 4096], BF16, name="xb16")

            def bf_feed(tag, src):
                # post-activation layers gather as bf16: quantize once here
                nc.vector.tensor_copy(ccb[:], src)
                nc.sync.dma_start(cc_in[tag][:], ccb[:])

            def bf_unload(tag, dstT):
                nc.sync.dma_start(
                    xb16[:].rearrange("p (k c) -> p k c", k=NCORES),
                    cc_out[tag][:].rearrange("(k p) c -> p k c", k=NCORES))
                # cast back to fp32r, split across DVE and ACT
                nc.vector.tensor_copy(dstT[:, 0:2048], xb16[:, 0:2048])
                nc.scalar.copy(dstT[:, 2048:4096], xb16[:, 2048:4096])

            # ---- weight loads (x_in + first-layer weights dispatched first so
            # the MLP can start immediately; bulk A-matrices follow)
            w1_sb = wp.tile([128, 1024], FR, name="w1_sb")
            nc.sync.dma_start(w1_sb[:], d_w1[:])
            b1_sb = wp.tile([128, 8], FP32, name="b1_sb")
            nc.sync.dma_start(
                b1_sb[:].rearrange("p (t o) -> p t o", t=8),
                d_b1[:].rearrange("(t p) o -> p t o", t=8))
            x_inT = t_b3[:, 0:CH]
            nc.sync.dma_start(x_inT, d_xin[:])
            w2_sb = t_b3[:, 4096:8192]
            nc.sync.dma_start(
                w2_sb.rearrange("p (k c) -> p k c", k=8),
                d_w2[:].rearrange("(k p) c -> p k c", k=8))
            b2_sb = wp.tile([128, 4], FP32, name="b2_sb")
            nc.sync.dma_start(
                b2_sb[:].rearrange("p (t o) -> p t o", t=4),
                d_b2[:].rearrange("(t p) o -> p t o", t=4))
            gw1_sb = wp.tile([128, 1024], FR, name="gw1_sb")
            nc.sync.dma_start(
                gw1_sb[:].rearrange("p (k c) -> p k c", k=4),
                d_gw1[:].rearrange("(k p) c -> p k c", k=4))
            gw2_sb = wp.tile([128, 256], FR, name="gw2_sb")
            nc.sync.dma_start(
                gw2_sb[:].rearrange("p (k c) -> p k c", k=2),
                d_gw2[:].rearrange("(k p) c -> p k c", k=2))
            bn1g_sb = wp.tile([128, 2], FP32, name="bn1g_sb")
            bn1b_sb = wp.tile([128, 2], FP32, name="bn1b_sb")
            nc.sync.dma_start(
                bn1g_sb[:].rearrange("p (t o) -> p t o", t=2),
                d_bn1g[:].rearrange("(t p) o -> p t o", t=2))
            nc.sync.dma_start(
                bn1b_sb[:].rearrange("p (t o) -> p t o", t=2),
                d_bn1b[:].rearrange("(t p) o -> p t o", t=2))
            bn2g_sb = wp.tile([128, 1], FP32, name="bn2g_sb")
            nc.sync.dma_start(bn2g_sb[:], d_bn2g[:])
            bn2b_sb = wp.tile([128, 1], FP32, name="bn2b_sb")
            nc.sync.dma_start(bn2b_sb[:], d_bn2b[:])
            swl_sb = wp.tile([128, 128], FR, name="swl_sb")
            nc.sync.dma_start(swl_sb[:], d_swl[:])
            swr_sb = wp.tile([128, 128], FR, name="swr_sb")
            nc.sync.dma_start(swr_sb[:], d_swr[:])
            sbl_sb = wp.tile([128, 1], FP32, name="sbl_sb")
            nc.sync.dma_start(sbl_sb[:], d_sbl[:])
            cw0_sb = wp.tile([128, 128], FR, name="cw0_sb")
            nc.sync.dma_start(cw0_sb[:], d_cw0[:])
            cw1_sb = wp.tile([128, 128], FR, name="cw1_sb")
            nc.sync.dma_start(cw1_sb[:], d_cw1[:])
            cb_sb = wp.tile([128, 1], FP32, name="cb_sb")
            nc.sync.dma_start(cb_sb[:], d_cb[:])
            gwva1_sb = wp.tile([128, 130], FR, name="gwva1_sb")
            nc.sync.dma_start(gwva1_sb[:], d_gwva1[:])
            vd1_sb = wp.tile([128, 2], FR, name="vd1_sb")
            nc.sync.dma_start(vd1_sb[:], d_vd1[:])
            g1b_sb = wp.tile([128, 1], FP32, name="g1b_sb")
            nc.sync.dma_start(g1b_sb[:], d_g1b[:])
            gwva2_sb = wp.tile([128, 130], FR, name="gwva2_sb")
            nc.sync.dma_start(gwva2_sb[:], d_gwva2[:])
            vd2_sb = wp.tile([128, 2], FR, name="vd2_sb")
            nc.sync.dma_start(vd2_sb[:], d_vd2[:])
            g2b_sb = wp.tile([128, 1], FP32, name="g2b_sb")
            nc.sync.dma_start(g2b_sb[:], d_g2b[:])


            # ===== MLP (node-sharded: this core's 512 nodes only) ===========
            # h1T [1024f, 512n] in t_b2[0:4096]; h2T [512f, 512n] in t_h2[0:2048]
            with tc.tile_pool(name="mlp_ps", bufs=2, space="PSUM") as mp:
                for t in range(8):
                    ps1 = mp.tile([128, 512], FP32, name="ps1", bufs=2)
                    nc.tensor.matmul(ps1[:], w1_sb[:, 128 * t:128 * (t + 1)],
                                     x_inT, start=True, stop=True)
                    nc.scalar.activation(
                        t_b2[:, 512 * t:512 * (t + 1)],
                        ps1[:], AF.Relu, bias=b1_sb[:, t:t + 1])
                for f2 in range(4):
                    ps2 = mp.tile([128, 512], FP32, name="ps2", bufs=2)
                    for k in range(8):
                        nc.tensor.matmul(
                            ps2[:],
                            w2_sb[:, 512 * k + 128 * f2:512 * k + 128 * f2 + 128],
                            t_b2[:, 512 * k:512 * (k + 1)],
                            start=(k == 0), stop=(k == 7))
                    nc.scalar.activation(
                        t_h2[:, 512 * f2:512 * (f2 + 1)],
                        ps2[:], AF.Relu, bias=b2_sb[:, f2:f2 + 1])

            # ===== GCN1 feature (local 512 nodes) + r-layout AllGather ======
            with tc.tile_pool(name="g1f_ps", bufs=2, space="PSUM") as gp:
                for nl in range(4):
                    psg = gp.tile([128, 256], FP32, name="psg", bufs=2)
                    for k in range(4):
                        nc.tensor.matmul(
                            psg[:], t_h2[:, 512 * k + 128 * nl:512 * k + 128 * nl + 128],
                            gw1_sb[:, 256 * k:256 * (k + 1)],
                            start=(k == 0), stop=(k == 3))
                    if nl % 2 == 0:
                        nc.vector.tensor_copy(
                            t_b2[:, 4096 + 256 * nl:4096 + 256 * (nl + 1)], psg[:])
                    else:
                        nc.scalar.copy(
                            t_b2[:, 4096 + 256 * nl:4096 + 256 * (nl + 1)], psg[:])
                nc.sync.dma_start(
                    cc_in_h[:].rearrange("(nl p) c -> p nl c", nl=4),
                    t_b2[:, 4096:5120].rearrange("p (nl c) -> p nl c", nl=4))
            nc.gpsimd.collective_compute(
                "AllGather", ALU.bypass, replica_groups=RG,
                ins=[cc_in_h[:].opt()], outs=[cc_out_h[:].opt()])
            # h_g1 for all 4096 nodes, r-layout tiles [128,256] into t_b2
            nc.sync.dma_start(
                t_b2[:, 0:8192].rearrange("p (rt c) -> p rt c", rt=NT),
                cc_out_h[:].rearrange("(rt p) c -> p rt c", rt=NT))
            if DEBUG_TAPS:
                nc.sync.dma_start(d_dbg["h"][:], t_b2[:, 0:8192])

            # pred weights into the h2 arena (free once GCN1 feature is done)
            pw_sb = t_h2[:, 8192:8192 + CSL + 1]
            nc.sync.dma_start(pw_sb, d_pw[:])

            # ============ GCN1 message (local chunk) + AllGather ============
            with tc.tile_pool(name="g1m_ps", bufs=1, space="PSUM") as gp:
                acc0 = gp.tile([128, 512], FP32, name="acc0")
                acc1 = gp.tile([128, 512], FP32, name="acc1")
                for rt in range(NT):
                    a_t = asp.tile([128, 512], FR, name="a_t", bufs=4)
                    nc.sync.dma_start(a_t[:], d_agcn[128 * rt:128 * (rt + 1), :])
                    mm(acc0[:], t_b2[:, 256 * rt:256 * rt + 128], a_t[:],
                       start=(rt == 0), stop=(rt == NT - 1))
                    mm(acc1[:], t_b2[:, 256 * rt + 128:256 * rt + 256],
                       a_t[:], start=(rt == 0), stop=(rt == NT - 1))
                nc.vector.tensor_copy(cc0[:], acc0[:])
                nc.vector.tensor_copy(cc1[:], acc1[:])
            nc.sync.dma_start(cc_in["gcn1"][0:128, :], cc0[:])
            nc.sync.dma_start(cc_in["gcn1"][128:256, :], cc1[:])
            nc.gpsimd.collective_compute(
                "AllGather", ALU.bypass, replica_groups=RG,
                ins=[cc_in["gcn1"][:].opt()], outs=[cc_out["gcn1"][:].opt()])
            g1view = cc_out["gcn1"][:].rearrange("(k f) c -> f k c", k=NCORES)
            nc.sync.dma_start(
                t_b3[:, 0:4096].rearrange("p (k c) -> p k c", k=NCORES),
                g1view[0:128])
            nc.scalar.dma_start(
                t_b3[:, 4096:8192].rearrange("p (k c) -> p k c", k=NCORES),
                g1view[128:256])

            # ===== BN1 + relu -> x3T (t_h2 blocks 0,1; h2T there is dead;
            # pw_sb at t_h2[8192:] must NOT be touched) =====================
            scratch = t_b2[:, 0:4096]
            for t in range(2):
                mt = t_b3[:, 4096 * t:4096 * (t + 1)]
                s, bpc = _batch_norm(nc, bnp, mt, scratch,
                                     bn1g_sb[:, t:t + 1], bn1b_sb[:, t:t + 1],
                                     1.0 / N)
                nc.scalar.activation(t_h2[:, 4096 * t:4096 * (t + 1)], mt,
                                     AF.Relu, bias=bpc[:], scale=s[:])
            if DEBUG_TAPS:
                nc.sync.dma_start(d_dbg["x3"][:], t_h2[:, 0:8192])

            # ============ GCN2 feature: h_g2 [n,128] in t_b2 ================
            with tc.tile_pool(name="g2f_ps", bufs=2, space="PSUM") as gp:
                for rt in range(NT):
                    psg = gp.tile([128, 128], FP32, name="psg2", bufs=2)
                    for k in range(2):
                        nc.tensor.matmul(
                            psg[:],
                            t_h2[:, 4096 * k + 128 * rt:4096 * k + 128 * rt + 128],
                            gw2_sb[:, 128 * k:128 * (k + 1)],
                            start=(k == 0), stop=(k == 1))
                    nc.vector.tensor_copy(t_b2[:, 128 * rt:128 * (rt + 1)], psg[:])

            # ============ GCN2 message + AllGather ==========================
            with tc.tile_pool(name="g2m_ps", bufs=1, space="PSUM") as gp:
                accm = gp.tile([128, 512], FP32, name="accm")
                for rt in range(NT):
                    a_t = asp.tile([128, 512], FR, name="a_t", bufs=4)
                    nc.sync.dma_start(a_t[:], d_agcn[128 * rt:128 * (rt + 1), :])
                    mm(accm[:], t_b2[:, 128 * rt:128 * (rt + 1)], a_t[:],
                       start=(rt == 0), stop=(rt == NT - 1))
                nc.vector.tensor_copy(cc0[:], accm[:])
            nc.sync.dma_start(cc_in["gcn2"][:], cc0[:])
            nc.gpsimd.collective_compute(
                "AllGather", ALU.bypass, replica_groups=RG,
                ins=[cc_in["gcn2"][:].opt()], outs=[cc_out["gcn2"][:].opt()])
            nc.sync.dma_start(
                t_b3[:, 0:4096].rearrange("p (k c) -> p k c", k=NCORES),
                cc_out["gcn2"][:].rearrange("(k p) c -> p k c", k=NCORES))

            # ============ BN2 + relu -> x4T (t_b3 block 1) + local ==========
            mt_a = t_b3[:, 0:4096]
            s2, bp2 = _batch_norm(nc, bnp, mt_a, scratch,
                                  bn2g_sb[:, 0:1], bn2b_sb[:, 0:1], 1.0 / N)
            x4T = t_b3[:, 4096:8192]
            nc.scalar.activation(x4T, mt_a, AF.Relu, bias=bp2[:], scale=s2[:])
            nc.scalar.activation(loc0[:], cc0[:], AF.Relu, bias=bp2[:], scale=s2[:])
            if DEBUG_TAPS:
                nc.sync.dma_start(d_dbg["x4"][:], x4T)

            # ============ SAGE ==============================================
            with tc.tile_pool(name="sage_ps", bufs=1, space="PSUM") as gp:
                for rt in range(NT):
                    psz = gp.tile([128, 128], FP32, name="psz", bufs=2)
                    nc.tensor.matmul(psz[:], x4T[:, 128 * rt:128 * (rt + 1)],
                                     swl_sb[:], start=True, stop=True)
                    if rt % 2 == 0:
                        nc.vector.tensor_copy(t_b2[:, 128 * rt:128 * (rt + 1)], psz[:])
                    else:
                        nc.scalar.copy(t_b2[:, 128 * rt:128 * (rt + 1)], psz[:])
                accs = gp.tile([128, 512], FP32, name="accs")
                for rt in range(NT):
                    a_t = asp.tile([128, 512], FR, name="a_t", bufs=4)
                    nc.sync.dma_start(a_t[:], d_asage[128 * rt:128 * (rt + 1), :])
                    mm(accs[:], t_b2[:, 128 * rt:128 * (rt + 1)], a_t[:],
                       start=(rt == 0), stop=False)
                nc.tensor.matmul(accs[:], swr_sb[:], loc0[:], start=False, stop=True)
                nc.scalar.activation(cc1[:], accs[:], AF.Relu, bias=sbl_sb[:])
            bf_feed("sage", cc1[:])
            nc.gpsimd.collective_compute(
                "AllGather", ALU.bypass, replica_groups=RG,
                ins=[cc_in["sage"][:].opt()], outs=[cc_out["sage"][:].opt()])
            x5T = t_h2[:, 0:4096]
            bf_unload("sage", x5T)
            if DEBUG_TAPS:
                nc.sync.dma_start(d_dbg["x5"][:], x5T)

            # ============ Cheb ==============================================
            with tc.tile_pool(name="cheb_ps", bufs=1, space="PSUM") as gp:
                for rt in range(NT):
                    psz = gp.tile([128, 128], FP32, name="psz1", bufs=2)
                    nc.tensor.matmul(psz[:], x5T[:, 128 * rt:128 * (rt + 1)],
                                     cw1_sb[:], start=True, stop=True)
                    if rt % 2 == 0:
                        nc.vector.tensor_copy(
                            t_b2[:, 4096 + 128 * rt:4096 + 128 * (rt + 1)], psz[:])
                    else:
                        nc.scalar.copy(
                            t_b2[:, 4096 + 128 * rt:4096 + 128 * (rt + 1)], psz[:])
                accc = gp.tile([128, 512], FP32, name="accc")
                for rt in range(NT):
                    a_t = asp.tile([128, 512], FR, name="a_t", bufs=4)
                    nc.sync.dma_start(a_t[:], d_acheb[128 * rt:128 * (rt + 1), :])
                    mm(accc[:], t_b2[:, 4096 + 128 * rt:4096 + 128 * (rt + 1)],
                       a_t[:], start=(rt == 0), stop=False)
                nc.tensor.matmul(accc[:], cw0_sb[:], cc1[:], start=False, stop=True)
                nc.scalar.activation(cc0[:], accc[:], AF.Relu, bias=cb_sb[:])
            bf_feed("cheb", cc0[:])
            nc.gpsimd.collective_compute(
                "AllGather", ALU.bypass, replica_groups=RG,
                ins=[cc_in["cheb"][:].opt()], outs=[cc_out["cheb"][:].opt()])
            x6T = t_b3[:, 0:4096]
            bf_unload("cheb", x6T)
            if DEBUG_TAPS:
                nc.sync.dma_start(d_dbg["x6"][:], x6T)

            # ============ GAT layers ========================================
            def gat_layer(xT, xloc, gwva_sb, vd_sb, gb_sb, h_base, out_loc, tag):
                with tc.tile_pool(name=f"{tag}_ps", bufs=1, space="PSUM") as gp:
                    # a_d broadcast first: it only needs the pre-gather local
                    # activations, so the e/exp chain can pipeline tile-by-tile
                    # with the feature loop below.
                    psd = gp.tile([1, 512], FP32, name="psd")
                    nc.tensor.matmul(psd[:], vd_sb[:], xloc[:], start=True, stop=True)
                    nc.vector.tensor_copy(ad_row[:], psd[:])
                    psb = gp.tile([128, 512], FP32, name="psb")
                    nc.tensor.matmul(psb[:], ones_row[:], ad_row[:],
                                     start=True, stop=True)
                    nc.vector.tensor_copy(adb[:], psb[:])
                    for rt in range(NT):
                        psh = gp.tile([128, 130], FP32, name="psh", bufs=2)
                        nc.tensor.matmul(psh[:], xT[:, 128 * rt:128 * (rt + 1)],
                                         gwva_sb[:], start=True, stop=True)
                        # gpsimd cannot read PSUM on TRN2 — keep these on
                        # ACT/DVE, alternating
                        if rt % 2 == 0:
                            nc.scalar.copy(
                                t_b2[:, h_base + 128 * rt:h_base + 128 * (rt + 1)],
                                psh[:, 0:128])
                        else:
                            nc.vector.tensor_copy(
                                t_b2[:, h_base + 128 * rt:h_base + 128 * (rt + 1)],
                                psh[:, 0:128])
                        nc.vector.tensor_copy(a_s_sb[:, rt:rt + 1], psh[:, 128:129])
                    accn = gp.tile([128, 512], FP32, name="accn")
                    accd = gp.tile([2, 512], FP32, name="accd")
                    for rt in range(NT):
                        # leaky-relu: ~7/16 of tiles on ACT (fused bias+lrelu),
                        # the rest on DVE (add + max(0.2x, x)) so ACT only has
                        # the exp on most tiles — balances the two engines.
                        e_t = ax.tile([128, 512], FP32, name="gat_et", bufs=2)
                        if rt % 16 < 10:
                            nc.scalar.activation(e_t[:], adb[:], AF.Lrelu,
                                                 bias=a_s_sb[:, rt:rt + 1], alpha=0.2)
                        else:
                            v_t = ax.tile([128, 512], FP32, name="gat_vt", bufs=2)
                            nc.vector.tensor_scalar_add(v_t[:], adb[:],
                                                        a_s_sb[:, rt:rt + 1])
                            nc.vector.scalar_tensor_tensor(e_t[:], v_t[:], 0.2,
                                                           v_t[:], ALU.mult, ALU.max)
                        x_t = ax.tile([128, 512], FP32, name="gat_xt", bufs=2)
                        nc.scalar.activation(x_t[:], e_t[:], AF.Exp)
                        m_t = asp.tile([128, 512], BF16, name="a_tb", bufs=8)
                        nc.sync.dma_start(m_t[:], d_mgat[128 * rt:128 * (rt + 1), :])
                        ab_t = ax.tile([128, 512], FR, name="gat_ab", bufs=2)
                        nc.vector.tensor_tensor(ab_t[:], x_t[:], m_t[:], ALU.mult)
                        nc.tensor.matmul(accn[:],
                                         t_b2[:, h_base + 128 * rt:h_base + 128 * (rt + 1)],
                                         ab_t[:], start=(rt == 0), stop=(rt == NT - 1))
                        nc.tensor.matmul(accd[:], ones_col[:], ab_t[:],
                                         start=(rt == 0), stop=(rt == NT - 1))
                    nc.vector.reciprocal(rec_row[:], accd[0:1, :])
                    psr = gp.tile([128, 512], FP32, name="psr")
                    nc.tensor.matmul(psr[:], ones_row[:], rec_row[:],
                                     start=True, stop=True)
                    nc.vector.tensor_copy(adb[:], accn[:])
                    prod = ax.tile([128, 512], FP32, name="gat_prod", bufs=2)
                    nc.vector.tensor_tensor(prod[:], adb[:], psr[:], ALU.mult)
                    r_t = ax.tile([128, 512], FP32, name="gat_et", bufs=2)
                    nc.scalar.activation(r_t[:], prod[:], AF.Relu, bias=gb_sb[:])
                    m_n = ax.tile([128, 512], FP32, name="gat_xt", bufs=2)
                    nc.vector.tensor_scalar(m_n[:], prod[:], gb_sb[:], 0.0,
                                            ALU.add, ALU.min)
                    e2 = ax.tile([128, 512], FP32, name="gat_ab", bufs=2)
                    nc.scalar.activation(e2[:], m_n[:], AF.Exp)
                    nc.vector.scalar_tensor_tensor(out_loc[:], e2[:], -1.0, r_t[:],
                                                   ALU.add, ALU.add)

            gat_layer(x6T, cc0, gwva1_sb, vd1_sb, g1b_sb, 0, cc1, "gat1")
            bf_feed("gat1", cc1[:])
            nc.gpsimd.collective_compute(
                "AllGather", ALU.bypass, replica_groups=RG,
                ins=[cc_in["gat1"][:].opt()], outs=[cc_out["gat1"][:].opt()])
            x7T = t_h2[:, 4096:8192]
            bf_unload("gat1", x7T)
            if DEBUG_TAPS:
                nc.sync.dma_start(d_dbg["x7"][:], x7T)

            gat_layer(x7T, cc1, gwva2_sb, vd2_sb, g2b_sb, 4096, cc0, "gat2")
            bf_feed("gat2", cc0[:])
            nc.gpsimd.collective_compute(
                "AllGather", ALU.bypass, replica_groups=RG,
                ins=[cc_in["gat2"][:].opt()], outs=[cc_out["gat2"][:].opt()])
            x8T = t_b3[:, 4096:8192]
            bf_unload("gat2", x8T)
            if DEBUG_TAPS:
                nc.sync.dma_start(d_dbg["x8"][:], x8T)

            # ============ pred: scores[n, CSL] = x8 @ pred_w slice ==========
            # 512-wide PSUM matmuls; fp32->bf16 casts rotate over DVE, ACT
            # and Pool into 2048-wide SBUF staging tiles, each flushed by one
            # wide DMA so the SP sequencer cost stays off the critical path.
            chunks = [(2048 * k, min(2048, CSL - 2048 * k)) for k in range(3)]
            with (
                tc.tile_pool(name="pred_ps", bufs=6, space="PSUM") as pp,
                tc.tile_pool(name="pred_out", bufs=2) as po,
            ):
                i = 0
                for nt in range(NT):
                    for ci, (c0, cw) in enumerate(chunks):
                        ot = po.tile([128, 2048], BF16, name="osb", bufs=2)
                        osb = ot[:, 0:cw]
                        for h in range(0, cw, 512):
                            hw = min(512, cw - h)
                            psp = pp.tile([128, 512], FP32, name="psp", bufs=6)
                            nc.tensor.matmul(psp[:, 0:hw],
                                             x8T[:, 128 * nt:128 * (nt + 1)],
                                             pw_sb[:, c0 + h:c0 + h + hw],
                                             start=True, stop=True)
                            # gpsimd cannot read PSUM on TRN2: DVE/ACT only
                            if i % 2 == 0:
                                nc.vector.tensor_copy(osb[:, h:h + hw], psp[:, 0:hw])
                            else:
                                nc.scalar.copy(osb[:, h:h + hw], psp[:, 0:hw])
                            i += 1
                        deng = nc.sync if (3 * nt + ci) % 2 == 0 else nc.scalar
                        deng.dma_start(
                            d_scores[128 * nt:128 * (nt + 1), c0:c0 + cw],
                            osb[:, 0:cw])
    return nc


_PROG = None


def _get_program():
    global _PROG
    if _PROG is None:
        _PROG = build_program()
    return _PROG


def host_prep(inputs):
    f32 = lambda a: np.ascontiguousarray(np.asarray(a), dtype=np.float32)
    bf = lambda a: np.ascontiguousarray(np.asarray(a, dtype=np.float32).astype(NPBF))
    ei = np.asarray(inputs["edge_index"])
    nx = np.asarray(inputs["node_x"])
    r = ei[0].astype(np.int64)
    c = ei[1].astype(np.int64)
    mult = np.bincount(r * N + c, minlength=N * N).reshape(N, N).astype(np.float32)

    deg = np.bincount(c, minlength=N).astype(np.float32) + 1.0
    dinv = deg ** -0.5
    a_gcn = mult * np.outer(dinv, dinv)
    idx = np.arange(N)
    a_gcn[idx, idx] += dinv * dinv

    cnt = np.bincount(c, minlength=N).astype(np.float32)
    a_sage = mult / np.maximum(cnt, 1.0)[None, :]

    deg0 = np.bincount(r, minlength=N).astype(np.float32)
    dinv0 = np.where(deg0 > 0, deg0 ** -0.5, 0.0).astype(np.float32)
    a_cheb = -(mult * np.outer(dinv0, dinv0))

    m_gat = mult
    m_gat[idx, idx] += 1.0

    a_gcn = a_gcn.astype(NPBF)
    a_sage = a_sage.astype(NPBF)
    a_cheb = a_cheb.astype(NPBF)
    m_gat = m_gat.astype(NPBF)

    ue = np.asarray(inputs["user_emb_w"])
    ie = np.asarray(inputs["item_emb_w"])
    x_in = np.concatenate([ue[nx[:, 0]], ie[nx[:, 1]]], axis=1)
    x_inT = np.asarray(x_in.T, dtype=np.float32).astype(NPBF)

    g1w = np.asarray(inputs["gat1_w"], dtype=np.float32)
    g2w = np.asarray(inputs["gat2_w"], dtype=np.float32)
    va1 = (g1w @ np.asarray(inputs["gat1_asrc"], dtype=np.float32)).reshape(128, 1)
    vd1 = (g1w @ np.asarray(inputs["gat1_adst"], dtype=np.float32)).reshape(128, 1)
    va2 = (g2w @ np.asarray(inputs["gat2_asrc"], dtype=np.float32)).reshape(128, 1)
    vd2 = (g2w @ np.asarray(inputs["gat2_adst"], dtype=np.float32)).reshape(128, 1)
    zc = np.zeros((128, 1), dtype=np.float32)
    gwva1 = bf(np.concatenate([g1w, va1, zc], axis=1))
    gwva2 = bf(np.concatenate([g2w, va2, zc], axis=1))
    vd1 = np.concatenate([vd1, zc], axis=1)
    vd2 = np.concatenate([vd2, zc], axis=1)

    pw_pad = np.zeros((128, NPAD), dtype=np.float32)
    pw_pad[:, :NCLS] = np.asarray(inputs["pred_w"], dtype=np.float32)
    pw_pad = pw_pad.astype(NPBF)

    common = {
        "w1": bf(inputs["mlp_w1"]),
        "b1": f32(np.asarray(inputs["mlp_b1"]).reshape(1024, 1)),
        "w2": bf(inputs["mlp_w2"]),
        "b2": f32(np.asarray(inputs["mlp_b2"]).reshape(512, 1)),
        "gcn_w1": bf(inputs["gcn_w1"]),
        "bn1_g": f32(np.asarray(inputs["bn1_g"]).reshape(256, 1)),
        "bn1_b": f32(np.asarray(inputs["bn1_b"]).reshape(256, 1)),
        "gcn_w2": bf(inputs["gcn_w2"]),
        "bn2_g": f32(np.asarray(inputs["bn2_g"]).reshape(128, 1)),
        "bn2_b": f32(np.asarray(inputs["bn2_b"]).reshape(128, 1)),
        "sage_wl": bf(inputs["sage_wl"]),
        "sage_bl": f32(np.asarray(inputs["sage_bl"]).reshape(128, 1)),
        "sage_wr": bf(inputs["sage_wr"]),
        "cheb_w0": bf(inputs["cheb_w0"]),
        "cheb_w1": bf(inputs["cheb_w1"]),
        "cheb_b": f32(np.asarray(inputs["cheb_b"]).reshape(128, 1)),
        "gwva1": gwva1, "vd1": bf(vd1),
        "g1b": f32(np.asarray(inputs["gat1_b"]).reshape(128, 1)),
        "gwva2": gwva2, "vd2": bf(vd2),
        "g2b": f32(np.asarray(inputs["gat2_b"]).reshape(128, 1)),
    }
    in_maps = []
    for k in range(NCORES):
        sl = slice(CH * k, CH * (k + 1))
        m = dict(common)
        m["x_inT"] = np.ascontiguousarray(x_inT[:, sl])
        m["a_gcn"] = np.ascontiguousarray(a_gcn[:, sl])
        m["a_sage"] = np.ascontiguousarray(a_sage[:, sl]).astype(NPBF)
        m["a_cheb"] = np.ascontiguousarray(a_cheb[:, sl]).astype(NPBF)
        m["m_gat"] = np.ascontiguousarray(m_gat[:, sl]).astype(NPBF)
        pwk = np.zeros((128, CSL + 1), dtype=np.float32)
        pwk[:, :CSL] = pw_pad[:, CSL * k:CSL * (k + 1)]
        m["pred_w"] = pwk
        in_maps.append(m)
    return in_maps


def kernel(**inputs):
    in_maps = host_prep(inputs)
    nc = _get_program()
    res = run_bass_kernel_spmd(nc, in_maps, list(range(NCORES)))
    out = np.concatenate(
        [np.asarray(res.results[k]["scores"], dtype=np.float32)
         for k in range(NCORES)], axis=1)[:, :NCLS]
    out = out + np.asarray(inputs["pred_b"], dtype=np.float32)[None, :]
    return np.ascontiguousarray(out, dtype=np.float32)
